# revision 33
# baseline (speedup 1.0000x reference)
"""BlockWiseEmbedding kernel for 8 Trainium2 NeuronCores.

Strategy (data-parallel tokens, replicated tables, bf16 end-to-end):
  - Host: route each token to its block via block_assignment/local_assignment
    (pure index bookkeeping on small int arrays), dedup rows per block, and
    deal each block's unique rows evenly across the 8 cores so every core
    gets an identical per-block row count (ceil(U_b/8), padded to a multiple
    of 128). Tables and transformer weights are staged to DRAM as bf16
    (block0 column-padded 64->128 so its gather element is 256B).
  - Device (identical SPMD program on all 8 cores): per block b, one
    gpsimd.dma_gather(transpose=True) pulls the routed rows straight into
    the matmul-ready layout [128 k-partitions, nk, tokens], so the PE
    consumes gathered tiles directly as stationary operands — no PE
    transposes and no PSUM round-trip. Matmuls accumulate over nk k-slices
    into PSUM [128 tok, 512], DVE/ACT cast PSUM f32 -> SBUF bf16, and one
    DMA per block writes the [rows_b, 512] bf16 result out.
  - Host: scatter per-core bf16 outputs back to token order, upcast to f32.

The embedding gather + per-block matmul runs on device; the host only
permutes small int32/int16 index arrays, casts dtypes, and reassembles.
"""

import os
import sys

import numpy as np

for _p in ("/opt/trn_rl_repo", "/root/.axon_site/_ro/trn_rl_repo"):
    if os.path.isdir(_p) and _p not in sys.path:
        sys.path.append(_p)

N_CORES = 8
OUT_DIM = 512
N_BLOCKS = 4

TRACE = False
# dummy PE transposes to keep the PE clock ramped while the GPSIMD library
# loads and the first gather's descriptors are generated
PE_WARMUP = 40

LAST_EXEC_NS = None
LAST_RESULTS = None

_CACHE = {}


def _cdiv(a, b):
    return -(-a // b)


def _build_program(sizes, table_rows, nb128, out_dim):
    """sizes are the EFFECTIVE (padded) row widths, multiples of 128."""
    import concourse.mybir as mybir
    from concourse import bacc, tile
    from concourse._compat import get_trn_type
    from concourse.library_config import mlp

    f32 = mybir.dt.float32
    bf16 = mybir.dt.bfloat16
    i16 = mybir.dt.int16
    nB = len(sizes)
    offs = [0]
    for n in nb128:
        offs.append(offs[-1] + n)
    tot = offs[-1]
    totcols = tot // 16

    # big blocks first: their gather desc-gen starts earliest and they carry
    # the most PE work
    border = sorted(range(nB), key=lambda b: -sizes[b])

    nc = bacc.Bacc(get_trn_type() or "TRN2", target_bir_lowering=False, num_swdge_queues=4)
    tabs = [
        nc.dram_tensor(f"block{b}", [table_rows[b], sizes[b]], bf16, kind="ExternalInput")
        for b in range(nB)
    ]
    trs = [
        nc.dram_tensor(f"trans{b}", [sizes[b], out_dim], bf16, kind="ExternalInput")
        for b in range(nB)
    ]
    idx = nc.dram_tensor("idx", [128, totcols], i16, kind="ExternalInput")
    identh = nc.dram_tensor("ident", [128, 128], bf16, kind="ExternalInput")
    out = nc.dram_tensor("out", [tot, out_dim], bf16, kind="ExternalOutput")

    nc.gpsimd.load_library(mlp)

    # engine-balance for PSUM->SBUF cast copies across DVE and ACT
    load = {"v": 0.0, "s": 0.0}

    def copy_psum(dst, src, elems):
        if load["v"] <= load["s"]:
            nc.vector.tensor_copy(dst, src)
            load["v"] += elems
        else:
            nc.scalar.copy(dst, src)
            load["s"] += elems * 1.7

    with tile.TileContext(nc) as tc:
        with (
            tc.tile_pool(name="const", bufs=1) as cpool,
            tc.tile_pool(name="gath", bufs=1) as gpool,
            tc.tile_pool(name="ot", bufs=2) as opool,
            tc.tile_pool(name="po", bufs=6, space="PSUM") as popool,
            tc.tile_pool(name="warm", bufs=1, space="PSUM") as wpool,
        ):
            # idx buffer first: the gathers depend on it
            idx_sb = cpool.tile([128, totcols], i16)
            nc.sync.dma_start(idx_sb[:], idx[:, :])
            ident = cpool.tile([128, 128], bf16)
            nc.sync.dma_start(ident[:], identh[:, :])
            # transformer weights on the scalar HWDGE queue so they don't
            # delay the idx load
            tr_sb = [None] * nB
            for b in border:
                s = sizes[b]
                nk = s // 128
                t = cpool.tile([128, nk, out_dim], bf16, tag=f"tr{b}")
                nc.scalar.dma_start(t[:], trs[b][:, :].rearrange("(k p) n -> p k n", p=128))
                tr_sb[b] = t

            if PE_WARMUP:
                warm = wpool.tile([128, 128], bf16, tag="warm")
                for _ in range(PE_WARMUP):
                    nc.tensor.transpose(warm[:], ident[:], ident[:])

            # transposed gathers, two per block (half each), spread over the 4
            # SWDGE queues so descriptor generation runs concurrently and the
            # PE can start on the first half of block3 as early as possible.
            # dst [128, nk, N] bf16 holds gathered rows k-slice-major, exactly
            # the matmul stationary layout.
            g_sb = {}  # (b, lo) -> tile
            halves = [None] * nB  # per block: list of (lo_col, n_idx)
            qn = 0
            for bi, b in enumerate(border):
                if nb128[b] == 0:
                    continue
                s = sizes[b]
                nk = s // 128
                N = nb128[b]
                if bi == 0 and N >= 512:
                    # biggest block: tiny leading parts so the PE starts ASAP
                    parts = [(0, 128), (128, 128), (256, N - 256)]
                elif bi == nB - 1 and N >= 512:
                    # last block: small trailing parts so the final output
                    # write flushes quickly
                    parts = [(0, N - 256), (N - 256, 128), (N - 128, 128)]
                elif N >= 256:
                    h = (N // 256) * 128
                    parts = [(0, h), (h, N - h)]
                else:
                    parts = [(0, N)]
                halves[b] = parts
                for lo, n in parts:
                    g = gpool.tile([128, nk, n], bf16, tag=f"g{b}_{lo}")
                    g_sb[(b, lo)] = g
                    nc.gpsimd.dma_gather(
                        g[:, :, :],
                        tabs[b][:, :],
                        idx_sb[:, (offs[b] + lo) // 16 : (offs[b] + lo + n) // 16],
                        n,
                        n,
                        s,
                        transpose=True,
                        queue_num=qn % 4,
                    )
                    qn += 1

            # out-write queue balancing: scalar's HWDGE queue already carries
            # the 1MB of transformer weights
            qbytes = {"sync": 0.07, "scalar": 1.05}
            npart = sum(len(h) for h in halves if h)
            pi = 0
            tail_alt = None
            for b in border:
                if nb128[b] == 0:
                    continue
                nk = sizes[b] // 128
                for lo, n in halves[b]:
                    if tail_alt is None and pi >= npart - 4:
                        tail_alt = 0
                    pi += 1
                    C = n // 128
                    g = g_sb[(b, lo)]
                    ot = opool.tile([128, C, out_dim], bf16, tag=f"ot{b}_{lo}")
                    for m in range(C):
                        po = popool.tile([128, out_dim], f32, tag="po")
                        for k in range(nk):
                            nc.tensor.matmul(
                                po[:, :],
                                g[:, k, m * 128 : (m + 1) * 128],
                                tr_sb[b][:, k, :],
                                start=(k == 0),
                                stop=(k == nk - 1),
                            )
                        copy_psum(ot[:, m, :], po[:, :], 128 * out_dim)
                    mb = n * out_dim * 2 / 1e6
                    if tail_alt is not None:
                        # trailing parts: strict alternation so the final
                        # issues and transfers run on both engines in parallel
                        eng = nc.sync if tail_alt % 2 == 0 else nc.scalar
                        tail_alt += 1
                    elif qbytes["sync"] <= qbytes["scalar"]:
                        eng = nc.sync
                        qbytes["sync"] += mb
                    else:
                        eng = nc.scalar
                        qbytes["scalar"] += mb
                    eng.dma_start(
                        out[offs[b] + lo : offs[b] + lo + n, :].rearrange(
                            "(m p) n -> p m n", p=128
                        ),
                        ot[:, :, :],
                    )

    nc.compile()
    return nc, offs, tot


def _route(src, block_assignment, local_assignment, table_rows):
    """Host-side token routing with row dedup. Each block's referenced table
    rows are deduplicated (np.unique -> sorted ascending, better HBM
    locality) and dealt evenly across cores. Returns per-core index buffers
    plus bookkeeping to reassemble outputs."""
    src_f = np.asarray(src).reshape(-1)
    ba = np.asarray(block_assignment)[src_f]
    la = np.asarray(local_assignment)[src_f]

    nb = [0] * N_BLOCKS
    nb128 = [0] * N_BLOCKS
    binfo = []
    for b in range(N_BLOCKS):
        toks = np.where(ba == b)[0]
        rows = np.clip(la[toks], 0, table_rows[b] - 1)
        urows, inv = np.unique(rows, return_inverse=True)
        binfo.append((toks, inv, urows))
        nb[b] = int(_cdiv(urows.size, N_CORES))
        nb128[b] = _cdiv(nb[b], 128) * 128

    offs = [0]
    for n in nb128:
        offs.append(offs[-1] + n)
    tot = offs[-1]
    totcols = tot // 16

    idx_bufs = np.zeros((N_CORES, 128, totcols), dtype=np.int16)
    for b in range(N_BLOCKS):
        toks, inv, urows = binfo[b]
        if urows.size == 0:
            continue
        for c in range(N_CORES):
            lo = c * nb[b]
            hi = min(urows.size, lo + nb[b])
            if hi <= lo:
                continue
            pad = np.zeros((nb128[b],), dtype=np.int16)
            pad[: hi - lo] = urows[lo:hi].astype(np.int16)
            # index j lives at [j % 16, j // 16], segment starts at column
            # offs[b] // 16; replicated to all 128 partitions (each Q7 core
            # pair reads its own copy)
            wrapped = pad.reshape(-1, 16).T  # [16, nb128/16]
            idx_bufs[c, :, offs[b] // 16 : offs[b] // 16 + nb128[b] // 16] = np.tile(
                wrapped, (8, 1)
            )
    return idx_bufs, binfo, tuple(nb), tuple(nb128), offs, tot


def _ensure_ntff_hook():
    """Register the axon NTFF profiling hook if the image's antenv lacks it."""
    try:
        from antenv.axon_hooks import get_axon_ntff_profile_hook  # noqa: F401

        return
    except ImportError:
        pass
    import types

    mod = types.ModuleType("antenv.axon_hooks")
    holder = {"h": None}
    mod.set_axon_ntff_profile_hook = lambda h: holder.__setitem__("h", h)
    mod.get_axon_ntff_profile_hook = lambda: holder["h"]
    sys.modules["antenv.axon_hooks"] = mod
    try:
        if "/root/.axon_site" not in sys.path:
            sys.path.append("/root/.axon_site")
        from trn_agent_boot.trn_boot import _ntff_profile_via_ctypes

        so = "/opt/axon/libaxon_pjrt.so"
        if os.path.exists(so):
            h = _ntff_profile_via_ctypes(so)
            if h is not None:
                mod.set_axon_ntff_profile_hook(h)
    except Exception:
        pass


def _to_bf16(x):
    import ml_dtypes

    return np.ascontiguousarray(np.asarray(x, dtype=np.float32).astype(ml_dtypes.bfloat16))


def _stage_trans(tr_bf):
    """[s, out_dim] -> [128, s//128, out_dim] with row (k*128+p) at [p, k]."""
    s, od = tr_bf.shape
    return np.ascontiguousarray(tr_bf.reshape(s // 128, 128, od).transpose(1, 0, 2))


def _unstage_out(arr):
    """[128, T, out_dim] partition-major -> [T*128, out_dim] token-major."""
    p, t, od = arr.shape
    return arr.transpose(1, 0, 2).reshape(t * p, od)


def kernel(
    src,
    block_assignment,
    local_assignment,
    block0,
    block1,
    block2,
    block3,
    trans0,
    trans1,
    trans2,
    trans3,
):
    global LAST_EXEC_NS, LAST_RESULTS
    import ml_dtypes
    from concourse.bass_utils import run_bass_kernel_spmd

    blocks = [np.asarray(x, dtype=np.float32) for x in (block0, block1, block2, block3)]
    trans = [np.asarray(x, dtype=np.float32) for x in (trans0, trans1, trans2, trans3)]
    orig_sizes = [b.shape[1] for b in blocks]
    table_rows = [b.shape[0] for b in blocks]
    src = np.asarray(src)

    # pad row widths up to a multiple of 128 (gather element >= 256B in bf16,
    # and the transposed layout wants whole 128-wide k-slices)
    sizes = [max(128, _cdiv(s, 128) * 128) for s in orig_sizes]
    blocks_bf = []
    trans_bf = []
    for b in range(N_BLOCKS):
        tbl, tr = blocks[b], trans[b]
        if sizes[b] != orig_sizes[b]:
            padc = sizes[b] - orig_sizes[b]
            tbl = np.concatenate([tbl, np.zeros((tbl.shape[0], padc), np.float32)], 1)
            tr = np.concatenate([tr, np.zeros((padc, tr.shape[1]), np.float32)], 0)
        blocks_bf.append(_to_bf16(tbl))
        trans_bf.append(_to_bf16(tr))

    idx_bufs, binfo, nb, nb128, offs, tot = _route(
        src, block_assignment, local_assignment, table_rows
    )

    key = (tuple(sizes), tuple(table_rows), nb128, "v2")
    if key not in _CACHE:
        _CACHE[key] = _build_program(sizes, table_rows, list(nb128), OUT_DIM)
    nc, _, _ = _CACHE[key]

    ident = np.eye(128, dtype=np.float32).astype(ml_dtypes.bfloat16)
    in_maps = []
    for c in range(N_CORES):
        m = {"idx": idx_bufs[c], "ident": ident}
        for b in range(N_BLOCKS):
            m[f"block{b}"] = blocks_bf[b]
            m[f"trans{b}"] = trans_bf[b]
        in_maps.append(m)

    if TRACE:
        _ensure_ntff_hook()
        import concourse.bass_utils as _bu

        if not getattr(_bu, "_upload_patched", False):
            _bu.upload_artifacts = lambda d: "local://" + d
            _bu._upload_patched = True
        try:
            res = run_bass_kernel_spmd(
                nc, in_maps, core_ids=list(range(N_CORES)), trace=True
            )
        except Exception:
            res = run_bass_kernel_spmd(
                nc, in_maps, core_ids=list(range(N_CORES)), trace=False
            )
    else:
        res = run_bass_kernel_spmd(
            nc, in_maps, core_ids=list(range(N_CORES)), trace=False
        )
    LAST_EXEC_NS = res.exec_time_ns
    LAST_RESULTS = res

    T = src.size
    out_flat = np.zeros((T, OUT_DIM), dtype=np.float32)
    all_out = np.stack(
        [res.results[c]["out"].astype(np.float32) for c in range(N_CORES)]
    )
    for b in range(N_BLOCKS):
        toks, inv, urows = binfo[b]
        if urows.size == 0:
            continue
        core = inv // nb[b]
        pos = inv % nb[b]
        out_flat[toks] = all_out[core, offs[b] + pos]
    return out_flat.reshape(src.shape + (OUT_DIM,))
